# revision 21
# baseline (speedup 1.0000x reference)
"""Causal self-attention (B=2, T=2048, C=1024, 16 heads) on 8 trn2 cores.

Sharding: 2 batches x 4 head-groups (4 heads each). Per core, t-chunk-major
pipeline: stream x columns per 512-wide chunk, project qkv for the chunk,
then run causal attention for the chunk immediately (causality: chunk c only
attends to k/v tiles <= c). Scores stay transposed [tk, tq]; y^T accumulates
in psum with a ones-column denominator row; normalization via K=1 broadcast
matmul + fast reciprocal. y^T is AllGathered across the 4 cores of each
batch per chunk (overlapped with the next chunk's attention), followed by a
transposed column-slice output projection per chunk. Host re-transposes and
concatenates the 8 [256, T] slices.
"""
import numpy as np
import ml_dtypes

import concourse.bacc as bacc
import concourse.mybir as mybir
import concourse.tile as tile
from concourse.bass_utils import run_bass_kernel_spmd

F32 = mybir.dt.float32
F32R = mybir.dt.float32r
BF16 = mybir.dt.bfloat16
EXP = mybir.ActivationFunctionType.Exp

B, T, C = 2, 2048, 1024
NH, HD = 16, 64
NCORES = 8
NG = 4            # head groups (tensor-parallel within a batch)
GC = 256          # features per group (4 heads * 64)
NFT = C // 128    # 8 feature tiles
NTT = T // 128    # 16 t tiles
NCH = T // 512    # 4 tq chunks

_nc_cache = {}


def build_nc():
    nc = bacc.Bacc("TRN2", target_bir_lowering=False, debug=False, num_devices=NCORES)
    xT = nc.dram_tensor("xT", [C, T], F32R, kind="ExternalInput")
    wq = nc.dram_tensor("wq", [C, GC], F32R, kind="ExternalInput")
    wk = nc.dram_tensor("wk", [C, GC], F32R, kind="ExternalInput")
    wv = nc.dram_tensor("wv", [C, GC], F32R, kind="ExternalInput")
    wpr = nc.dram_tensor("wpr", [GC, C], F32R, kind="ExternalInput")
    tri = nc.dram_tensor("tri", [128, 128], F32R, kind="ExternalInput")
    ones = nc.dram_tensor("ones", [128, 64], F32R, kind="ExternalInput")
    outP = nc.dram_tensor("outP", [C, T], F32, kind="ExternalOutput")

    with tile.TileContext(nc) as tc:
        with (
            tc.tile_pool(name="xc", bufs=1) as xcp,        # streamed x chunks
            tc.tile_pool(name="wpool", bufs=1) as wpool,
            tc.tile_pool(name="qk", bufs=1) as qkpool,
            tc.tile_pool(name="vpool", bufs=1) as vpool,
            tc.tile_pool(name="work", bufs=1) as work,
            tc.tile_pool(name="ytpool", bufs=1) as ytpool,
            tc.tile_pool(name="dram", bufs=1, space="DRAM") as dram,
            tc.tile_pool(name="psum", bufs=1, space="PSUM") as ps,
        ):
            # ---------------- loads: tiny consts, then x chunk0 + wq interleaved ----------------
            tri_sb = wpool.tile([128, 128], F32R, name="tri_sb")
            nc.sync.dma_start(tri_sb[:], tri[:])
            ones_sb = wpool.tile([128, 64], F32R, name="ones_sb")
            nc.sync.dma_start(ones_sb[:], ones[:])

            xc_tiles = {}

            def load_xc(c):
                xc = [
                    xcp.tile([128, 512], F32R, tag="xc", bufs=16, name=f"xc{c}_{i}")
                    for i in range(NFT)
                ]
                for i in range(NFT):
                    nc.sync.dma_start(
                        xc[i][:], xT[128 * i : 128 * (i + 1), 512 * c : 512 * (c + 1)]
                    )
                    if c == 0:
                        nc.sync.dma_start(wqt[i][:], wq[128 * i : 128 * (i + 1), :])
                xc_tiles[c] = xc

            wqt = [wpool.tile([128, GC], F32R, tag="wq", bufs=8, name=f"wqt{i}") for i in range(NFT)]
            wkt = [wpool.tile([128, GC], F32R, tag="wk", bufs=8, name=f"wkt{i}") for i in range(NFT)]
            wvt = [wpool.tile([128, GC], F32R, tag="wv", bufs=8, name=f"wvt{i}") for i in range(NFT)]
            wprt = [wpool.tile([128, C], F32R, tag="wpr", bufs=2, name=f"wprt{f}") for f in range(2)]

            load_xc(0)
            for i in range(NFT):
                nc.sync.dma_start(wkt[i][:], wk[128 * i : 128 * (i + 1), :])
            for i in range(NFT):
                nc.sync.dma_start(wvt[i][:], wv[128 * i : 128 * (i + 1), :])
            for f in range(2):
                nc.sync.dma_start(wprt[f][:], wpr[128 * f : 128 * (f + 1), :])

            # persistent per-core tensors
            qT = [qkpool.tile([128, T], F32R, tag="qT", bufs=2, name=f"qT{p}") for p in range(2)]
            kT = [qkpool.tile([128, T], F32R, tag="kT", bufs=2, name=f"kT{p}") for p in range(2)]
            vb = [vpool.tile([128, 260], F32R, tag="v", bufs=NTT, name=f"vb{tt}") for tt in range(NTT)]
            yT_sb = [
                ytpool.tile([64, T], F32R, tag="yt", bufs=4, name=f"yTsb{ph}")
                for ph in range(4)
            ]
            def proj_partial(c):
                # pack the 4 heads' normalized y^T for chunk c into [128, 512]
                # tiles (cross-partition move => DMA), then project against the
                # row-slice of W_proj; host sums partials across the 4 cores.
                ypk = [
                    work.tile([128, 512], F32R, tag="ypk", bufs=4, name=f"ypk{c}_{f}")
                    for f in range(2)
                ]
                for pp in range(2):
                    for h in range(2):
                        nc.gpsimd.dma_start(
                            ypk[pp][64 * h : 64 * (h + 1), :],
                            yT_sb[2 * pp + h][:, 512 * c : 512 * (c + 1)],
                        )
                for u in range(NFT):
                    opp = ps.tile([128, 512], F32, tag="mix", bufs=2, name=f"opp{c}{u}")
                    for f in range(2):
                        nc.tensor.matmul(
                            opp[:],
                            wprt[f][:, 128 * u : 128 * (u + 1)],
                            ypk[f][:],
                            start=(f == 0),
                            stop=(f == 1),
                        )
                    osb = work.tile([128, 512], F32, tag="osb", bufs=3, name=f"osb{c}{u}")
                    nc.vector.tensor_copy(osb[:], opp[:])
                    nc.gpsimd.dma_start(
                        outP[128 * u : 128 * (u + 1), 512 * c : 512 * (c + 1)], osb[:]
                    )

            pending_norm = []

            def flush_norms():
                for (p_, c_, h_, yrw) in pending_norm:
                    bc = ps.tile([64, 512], F32, tag="mix", bufs=2, name=f"bc{p_}{c_}{h_}")
                    nc.tensor.matmul(
                        bc[:], ones_sb[64:65, :], yrw[64:65, :], start=True, stop=True
                    )
                    rcp = work.tile([64, 512], F32, tag="rcp", bufs=2, name=f"rcp{p_}{c_}{h_}")
                    nc.vector.reciprocal_approx_fast(rcp[:], bc[:])
                    nc.vector.tensor_mul(
                        yT_sb[2 * p_ + h_][:, 512 * c_ : 512 * (c_ + 1)],
                        yrw[0:64, :],
                        rcp[:],
                    )
                pending_norm.clear()

            # ---------------- per-chunk pipeline ----------------
            for c in range(NCH):
                xc = xc_tiles[c]
                # qkv for this chunk
                for p in range(2):
                    qps = ps.tile([128, 512], F32, tag="mix", bufs=2, name=f"qps{p}_{c}")
                    for i in range(NFT):
                        nc.tensor.matmul(
                            qps[:],
                            wqt[i][:, 128 * p : 128 * (p + 1)],
                            xc[i][:],
                            start=(i == 0),
                            stop=(i == NFT - 1),
                        )
                    nc.vector.tensor_copy(qT[p][:, 512 * c : 512 * (c + 1)], qps[:])
                    kps = ps.tile([128, 512], F32, tag="mix", bufs=2, name=f"kps{p}_{c}")
                    for i in range(NFT):
                        nc.tensor.matmul(
                            kps[:],
                            wkt[i][:, 128 * p : 128 * (p + 1)],
                            xc[i][:],
                            start=(i == 0),
                            stop=(i == NFT - 1),
                        )
                    nc.vector.tensor_copy(kT[p][:, 512 * c : 512 * (c + 1)], kps[:])
                for ttl in range(4):
                    tt = 4 * c + ttl
                    vps = ps.tile([128, 256], F32, tag="mix", bufs=2, name=f"vps{tt}")
                    for i in range(NFT):
                        nc.tensor.matmul(
                            vps[:],
                            xc[i][:, 128 * ttl : 128 * (ttl + 1)],
                            wvt[i][:],
                            start=(i == 0),
                            stop=(i == NFT - 1),
                        )
                    nc.sync.dma_start(
                        vb[tt][:].rearrange("p (s c) -> p s c", s=4)[:, :, 64:65],
                        ones_sb[:, 0:4].rearrange("p (s o) -> p s o", o=1),
                    )
                    nc.vector.tensor_copy(
                        vb[tt][:].rearrange("p (s c) -> p s c", s=4)[:, :, 0:64],
                        vps[:].rearrange("p (s c) -> p s c", s=4),
                    )
                # prefetch next chunk's x right behind this chunk's compute wave
                if c + 1 < NCH:
                    load_xc(c + 1)

                flush_norms()
                if c >= 1:
                    proj_partial(c - 1)

                # attention for this chunk; j-loop software-pipelined one
                # stage so PE runs scores(j+1) while ACT computes exp(j)
                for p in range(2):
                    yta = [
                        ps.tile([65, 512], F32, tag=f"yta{h}", bufs=1, name=f"yta{p}{c}{h}")
                        for h in range(2)
                    ]

                    def scores_exp(j):
                        d = j - 4 * c
                        off = 128 * max(d, 0)
                        sps = ps.tile([128, 1024], F32, tag="sps", bufs=2, name=f"sps{p}{c}{j}")
                        for h in range(2):
                            nc.tensor.matmul(
                                sps[:, 512 * h + off : 512 * (h + 1)],
                                kT[p][64 * h : 64 * (h + 1), 128 * j : 128 * (j + 1)],
                                qT[p][64 * h : 64 * (h + 1), 512 * c + off : 512 * (c + 1)],
                                start=True,
                                stop=True,
                            )
                        es = work.tile([128, 1024], F32R, tag="es", bufs=4, name=f"es{p}{c}{j}")
                        nc.scalar.activation(
                            es[:].rearrange("p (g n) -> p g n", g=2)[:, :, off:512],
                            sps[:].rearrange("p (g n) -> p g n", g=2)[:, :, off:512],
                            EXP,
                            scale=0.125,
                        )
                        if d >= 0:
                            for h in range(2):
                                nc.vector.tensor_mul(
                                    es[:, 512 * h + off : 512 * h + off + 128],
                                    es[:, 512 * h + off : 512 * h + off + 128],
                                    tri_sb[:],
                                )
                        return es

                    def av(j, es):
                        d = j - 4 * c
                        off = 128 * max(d, 0)
                        for h in range(2):
                            hs = 2 * p + h
                            nc.tensor.matmul(
                                yta[h][:, off:512],
                                vb[j][:, 65 * hs : 65 * hs + 65],
                                es[:, 512 * h + off : 512 * (h + 1)],
                                start=(j == 0),
                                stop=(j == 4 * c + 3),
                            )

                    prev = None
                    for j in range(4 * c + 4):
                        es = scores_exp(j)
                        if prev is not None:
                            av(*prev)
                        prev = (j, es)
                    av(*prev)
                    # evacuate yta now (releases psum); defer the normalize
                    # (bc matmul + recip + mul) so PE is not stalled here
                    for h in range(2):
                        yrw = work.tile([65, 512], F32R, tag="yrw", bufs=6, name=f"yrw{p}{c}{h}")
                        nc.vector.tensor_copy(yrw[:], yta[h][:])
                        pending_norm.append((p, c, h, yrw))

            flush_norms()
            proj_partial(NCH - 1)

    nc.compile()
    return nc


def _get_nc():
    if "nc" not in _nc_cache:
        _nc_cache["nc"] = build_nc()
    return _nc_cache["nc"]


def _in_maps(x, W_attn, W_proj):
    tri = np.triu(np.ones((128, 128), np.float32))
    ones = np.ones((128, 64), np.float32)
    maps = []
    for core in range(NCORES):
        b, g = core // NG, core % NG
        lo = g * GC
        maps.append(
            {
                "xT": np.ascontiguousarray(x[b].T),
                "wq": np.ascontiguousarray(W_attn[:, lo : lo + GC]),
                "wk": np.ascontiguousarray(W_attn[:, C + lo : C + lo + GC]),
                "wv": np.ascontiguousarray(W_attn[:, 2 * C + lo : 2 * C + lo + GC]),
                "wpr": np.ascontiguousarray(W_proj[lo : lo + GC, :]),
                "tri": tri,
                "ones": ones,
            }
        )
    return maps


def kernel(x, W_attn, W_proj, **run_kwargs):
    x = np.asarray(x, np.float32)
    W_attn = np.asarray(W_attn, np.float32)
    W_proj = np.asarray(W_proj, np.float32)
    nc = _get_nc()
    res = run_bass_kernel_spmd(
        nc, _in_maps(x, W_attn, W_proj), core_ids=list(range(NCORES)), **run_kwargs
    )
    out = np.empty((B, T, C), np.float32)
    for b in range(B):
        acc = res.results[NG * b]["outP"].copy()
        for g in range(1, NG):
            acc += res.results[NG * b + g]["outP"]
        out[b] = acc.T
    if run_kwargs:
        kernel.last_result = res
    return out
